# revision 22
# baseline (speedup 1.0000x reference)
"""Trainium2 Bass kernel for the ChebyshevBasis problem.

Computes, for x:[8192,512], coeffs:[512,512,16], base_weight:[512,512]:
    t = tanh(x); basis_n = T_n(t) (Chebyshev); out = einsum('bfn,fon->bo') + x@base_weight
Returns (out:[8192,512] f32, kl=zeros(1)).

Strategy (8 NeuronCores, data-parallel over batch):
  - Each core handles 1024 batch rows; weights replicated (loaded once to SBUF).
  - The contraction is one big matmul [1024, K=15*512+512] @ [K, 512] done as
    64 K-tiles of 128 on the tensor engine in float32r (full-rate fp32).
  - x is pre-transposed on the host so the contraction dim (features) lands on
    SBUF partitions with no on-chip transposes.
  - Basis terms in [feature, batch] layout:
      * T_1 = tanh(x^T) on ScalarE
      * even T_2k = 2*T_k^2 - 1 (ScalarE Square + fused VectorE tensor_scalar)
      * odd  T_n  = 2*T_1*T_{n-1} - T_{n-2} (VectorE tensor_tensor +
        scalar_tensor_tensor), all running concurrently with the PE stream
      * T_0 == 1 is folded into a per-output bias = sum_f coeffs[f,o,0] (host
        weight repack), added during the PSUM->SBUF copy.
"""

import numpy as np

B, F, O, DEG = 8192, 512, 512, 15
N_CORES = 8
BSH = B // N_CORES          # 1024 batch rows per core
CB = 256                    # batch chunk (2 blocks of 128)
NBLK = CB // 128            # 2
NCHUNK = BSH // CB          # 4
NT = DEG                    # 15 stored terms, n = 1..15
KT = NT * 4 + 4             # 64 k-tiles of 128 (15 terms * 4 fblocks + residual)

_CACHE = {}


def _build_bass(repeat=1):
    import concourse.bacc as bacc
    import concourse.mybir as mybir
    from concourse.tile import TileContext

    AF = mybir.ActivationFunctionType
    OP = mybir.AluOpType
    F32 = mybir.dt.float32
    F32R = mybir.dt.float32r

    nc = bacc.Bacc("TRN2", target_bir_lowering=False, debug=False,
                   num_devices=N_CORES)
    xt_ext = nc.declare_dram_parameter("xt", [F, BSH], F32, isOutput=False)
    w_ext = nc.declare_dram_parameter("w", [KT, 128, O], F32, isOutput=False)
    bias_ext = nc.declare_dram_parameter("bias", [128, O], F32, isOutput=False)
    out_ext = nc.declare_dram_parameter("out", [BSH, O], F32, isOutput=True)

    with TileContext(nc) as tc:
        from contextlib import ExitStack
        es = ExitStack()
        with es:
            wpool = es.enter_context(tc.tile_pool(name="w", bufs=1))
            fixed = es.enter_context(tc.tile_pool(name="fixed", bufs=1))
            xtrp = es.enter_context(tc.tile_pool(name="xtr", bufs=3))
            termp = es.enter_context(tc.tile_pool(name="term", bufs=14))
            outp = es.enter_context(tc.tile_pool(name="out", bufs=2))
            ps_acc = es.enter_context(tc.tile_pool(name="ps_acc", bufs=6, space="PSUM"))

            # warm the ACT table set (tanh/square) while DMAs run
            dummy = fixed.tile([128, 1], F32, tag="dummy")
            nc.vector.memset(dummy[:], 0.0)
            nc.scalar.activation(dummy[:], dummy[:], AF.Tanh)
            # warm the PE HAM clock gate (~3.4us of activity -> 2.4 GHz)
            # with tiny matmuls while the first x/W DMAs are in flight
            ps_warm = es.enter_context(tc.tile_pool(name="ps_warm", bufs=1,
                                                    space="PSUM"))
            warm = ps_warm.tile([1, 1], F32, tag="warm")
            for i in range(32):
                nc.tensor.matmul(warm[:], dummy[:], dummy[:],
                                 start=True, stop=True)
            bias_sb = fixed.tile([128, O], F32, tag="bias")

            # resident weights: 8 groups of 8 k-tiles, one ~2MB DMA each so
            # the transfer spreads across all 16 SDMA engines at line rate
            # residual weights (kt 60-63) load first: the residual matmuls
            # depend only on x^T, so they fill the PE while the 17MB of
            # basis weights stream in
            wres = wpool.tile([128, 4 * O], F32R, tag="wres")
            for h in range(4):
                nc.scalar.dma_start(out=wres[:, h * O:(h + 1) * O],
                                    in_=w_ext[60 + h].bitcast(F32R))
            wt = []

            def wslice(kt):
                if kt >= 60:
                    return wres[:, (kt - 60) * O:(kt - 59) * O]
                g, i = divmod(kt, 8)
                return wt[g][:, i * O:(i + 1) * O]

            def prologue(c, split=1):
                """Load x^T chunk c (f32r) and compute T1; returns (xTr, T1).

                split>1 loads/activates in fb-slices so the first matmuls
                (which read only the first fb block) can start sooner."""
                xTr = xtrp.tile([128, 4 * CB], F32R, tag="xtr", name=f"xtr{c}")
                t1 = termp.tile([128, 4 * CB], F32R, tag="term", name=f"t1_{c}")
                w = 4 // split
                for h in range(split):
                    src = xt_ext[h * w * 128:(h + 1) * w * 128,
                                 c * CB:(c + 1) * CB] \
                        .rearrange("(fb p) b -> p fb b", p=128).bitcast(F32R)
                    dst = xTr[:, h * w * CB:(h + 1) * w * CB] \
                        .rearrange("p (fb b) -> p fb b", fb=w)
                    (nc.sync if c == 0 else nc.gpsimd).dma_start(out=dst, in_=src)
                    nc.scalar.activation(t1[:, h * w * CB:(h + 1) * w * CB],
                                         xTr[:, h * w * CB:(h + 1) * w * CB],
                                         AF.Tanh)
                return xTr, t1

            # chunk groups: first two chunks software-pipelined together so
            # the PE k-stream consumes resident W groups fast enough to hide
            # the initial 17MB weight load; later chunks run one at a time.
            groups = []
            for r in range(repeat):
                groups += [[r * NCHUNK, r * NCHUNK + 1],
                           [r * NCHUNK + 2], [r * NCHUNK + 3]]
            flat = [c for g in groups for c in g]
            pro = {}

            def ensure_pro(ci, split=1):
                if ci < len(flat) and ci not in pro:
                    pro[ci] = prologue(flat[ci] % NCHUNK, split=split)

            ensure_pro(0, split=4)
            ensure_pro(1)
            nc.gpsimd.dma_start(out=bias_sb[:], in_=bias_ext[:])
            # basis weights stream after chunk-0's x slices on the sync ring
            for g in range(8):
                nk = 8 if g < 7 else 4      # group 7 holds only T15 (kt 56-59)
                wg = wpool.tile([128, nk * O], F32R, tag=f"w{g}", name=f"wg{g}")
                nsub = 4 if g == 0 else 2   # finer first DMAs: PE starts sooner
                kw = nk // nsub
                for h in range(nsub):
                    wsrc = w_ext[g * 8 + kw * h:g * 8 + kw * (h + 1)] \
                        .rearrange("k p o -> p k o")
                    wdst = wg[:, kw * h * O:kw * (h + 1) * O] \
                        .rearrange("p (k o) -> p k o", k=kw)
                    nc.sync.dma_start(out=wdst, in_=wsrc.bitcast(F32R))
                wt.append(wg)
            done = 0  # chunks fully emitted
            for gi, grp in enumerate(groups):
                k = len(grp)
                st = {}
                for i, cf in enumerate(grp):
                    ensure_pro(done + i)
                    xTr, t1 = pro[done + i]
                    acc = [ps_acc.tile([128, O], F32, tag="acc",
                                       name=f"acc{cf}_{j}") for j in range(NBLK)]
                    st[cf] = {"xTr": xTr, "T": {1: t1}, "acc": acc}

                def emit_mms(cf, n):
                    ti = n - 1
                    tile, acc = st[cf]["T"][n], st[cf]["acc"]
                    for fb in range(4):
                        kt = ti * 4 + fb
                        for j in range(NBLK):
                            nc.tensor.matmul(
                                acc[j][:],
                                tile[:, fb * CB + j * 128: fb * CB + j * 128 + 128],
                                wslice(kt), start=False, stop=False)

                # residual k-tiles first: they only need x^T, so they give
                # the PE work while basis weights / T1 are still arriving
                for cf in grp:
                    xTr, acc = st[cf]["xTr"], st[cf]["acc"]
                    for fb in range(4):
                        kt = NT * 4 + fb
                        for j in range(NBLK):
                            nc.tensor.matmul(
                                acc[j][:],
                                xTr[:, fb * CB + j * 128: fb * CB + j * 128 + 128],
                                wslice(kt), start=(fb == 0), stop=False)
                for cf in grp:
                    emit_mms(cf, 1)
                for n in range(2, NT):
                    for cf in grp:
                        T = st[cf]["T"]
                        tn = termp.tile([128, 4 * CB], F32R, tag="term",
                                        name=f"t{n}_{cf}")
                        if n % 2 == 0:
                            # T_2k = 2*T_k^2 - 1, in place
                            nc.scalar.activation(tn[:], T[n // 2][:], AF.Square)
                            nc.vector.tensor_scalar(tn[:], tn[:], 2.0, 1.0,
                                                    OP.mult, OP.subtract)
                        else:
                            # T_n = (T_{n-1}*2)*T_1 - T_{n-2}, in place
                            nc.vector.scalar_tensor_tensor(
                                tn[:], T[n - 1][:], 2.0, T[1][:],
                                OP.mult, OP.mult)
                            nc.vector.tensor_tensor(tn[:], tn[:], T[n - 2][:],
                                                    OP.subtract)
                        T[n] = tn
                        emit_mms(cf, n)
                    if n == 6:
                        for i in range(len(groups[gi + 1]) if gi + 1 < len(groups) else 0):
                            ensure_pro(done + k + i)

                # final term T15: finish bank j then immediately bias-add +
                # store it, so the epilogue of bank j overlaps the final
                # matmuls of bank j+1
                for cf in grp:
                    T = st[cf]["T"]
                    tn = termp.tile([128, 4 * CB], F32R, tag="term",
                                    name=f"t{NT}_{cf}")
                    nc.vector.scalar_tensor_tensor(
                        tn[:], T[NT - 1][:], 2.0, T[1][:], OP.mult, OP.mult)
                    nc.vector.tensor_tensor(tn[:], tn[:], T[NT - 2][:],
                                            OP.subtract)
                    T[NT] = tn
                    acc = st[cf]["acc"]
                    for j in range(NBLK):
                        for fb in range(4):
                            kt = (NT - 1) * 4 + fb
                            nc.tensor.matmul(
                                acc[j][:],
                                tn[:, fb * CB + j * 128: fb * CB + j * 128 + 128],
                                wslice(kt), start=False, stop=(fb == 3))
                        ob = outp.tile([128, O], F32, tag="ob",
                                       name=f"ob{cf}_{j}")
                        nc.vector.tensor_tensor(ob[:], acc[j][:],
                                                bias_sb[:], OP.add)
                        bb = (cf % NCHUNK) * NBLK + j
                        nc.gpsimd.dma_start(
                            out=out_ext[bb * 128:(bb + 1) * 128, :], in_=ob[:])
                done += k

    nc.compile()
    return nc


def _repack_weights(coeffs, base_weight):
    w = np.empty((KT, 128, O), dtype=np.float32)
    for n in range(1, NT + 1):
        for fb in range(4):
            w[(n - 1) * 4 + fb] = coeffs[fb * 128:(fb + 1) * 128, :, n]
    for fb in range(4):
        w[NT * 4 + fb] = base_weight[fb * 128:(fb + 1) * 128, :]
    bias = coeffs[:, :, 0].sum(axis=0, dtype=np.float64).astype(np.float32)
    bias_rep = np.ascontiguousarray(np.broadcast_to(bias[None, :], (128, O)))
    return w, bias_rep


def _make_in_maps(x, coeffs, base_weight):
    w, bias_rep = _repack_weights(coeffs, base_weight)
    xt = np.ascontiguousarray(x.T)          # [F, B]
    return [
        {"xt": np.ascontiguousarray(xt[:, c * BSH:(c + 1) * BSH]),
         "w": w, "bias": bias_rep}
        for c in range(N_CORES)
    ]


def kernel(x, coeffs, base_weight, _run_kwargs=None):
    from concourse.bass_utils import run_bass_kernel_spmd

    x = np.asarray(x, dtype=np.float32)
    coeffs = np.asarray(coeffs, dtype=np.float32)
    base_weight = np.asarray(base_weight, dtype=np.float32)

    if "nc" not in _CACHE:
        _CACHE["nc"] = _build_bass()
    nc = _CACHE["nc"]

    in_maps = _make_in_maps(x, coeffs, base_weight)
    res = run_bass_kernel_spmd(nc, in_maps, list(range(N_CORES)),
                               **(_run_kwargs or {}))
    out = np.concatenate([res.results[c]["out"] for c in range(N_CORES)], axis=0)
    kl = np.zeros((1,), dtype=np.float32)
    if _run_kwargs:
        _CACHE["last_results"] = res
    return out, kl
